# revision 1
# baseline (speedup 1.0000x reference)
"""Ising log-energy kernel for Trainium2 (8 NeuronCores).

Reference computation (B=512 samples, N=4096 spins on a 64x64 grid):
    e[b] = sum_i u[i]*x[b,i] + sum_{i<j} (binary*mask)[i,j]*x[b,i]*x[b,j]

The mask is the nearest-neighbor upper-triangular grid mask: the only
nonzeros of w = binary*mask sit on the +1 and +64 off-diagonals. So

    e[b] = sum_i u[i]*x[b,i] + wr[i]*x[b,i]*x[b,i+1] + wd[i]*x[b,i]*x[b,i+64]

with wr/wd the masked diagonals of `binary`. That's O(B*N) work.

Distribution: tensor-parallel over sites. Core c owns sites
[c*512, c*512+512) for all 512 samples; partial energies are summed on
the host. On-device layout is site-major ([site, batch] = [partition,
free]) so per-site weights ride as matmul lhsT columns and the site sum
is the PE's partition contraction:

  per 128-site chunk k (4 per core), T0/T1/T64 = x rows shifted 0/+1/+64
  (bf16 — exact for +-1 spins):
    DVE : q1  = T0*T1,  q64 = T0*T64          (bf16, exact)
    PE  : acc[2,512] += [w_hi|w_lo].T @ {T0, q1, q64}

  Each fp32 weight vector is split w = hi + lo into two bf16 columns
  (relative error ~2^-17); both columns contract in a single matmul and
  the host adds the two PSUM rows.
"""

import os
from contextlib import ExitStack
import sys

import numpy as np

for _p in ("/opt/trn_rl_repo", "/root/.axon_site/_ro/trn_rl_repo"):
    if os.path.isdir(_p) and _p not in sys.path:
        sys.path.insert(0, _p)

import ml_dtypes

import concourse.bass as bass
import concourse.mybir as mybir
from concourse.bass_utils import run_bass_kernel_spmd


N = 4096          # total spins (64x64 grid)
NG = 64           # grid side (down-neighbor stride)
B = 512           # batch
NCORES = 8
S = N // NCORES   # sites per core = 512
PAD = NG          # extra x rows needed for the +64 shift
NCHUNK = S // 128  # 128-site chunks per core = 4

FP32 = mybir.dt.float32
BF16 = mybir.dt.bfloat16


def _build_bass():
    """Raw Bass (no Tile): the local walrus build only encodes ONE sync
    wait per instruction, so all waits are single cumulative-value waits
    on one of three counting semaphores (dma/dve/pe)."""
    nc = bass.Bass()
    xt = nc.declare_dram_parameter("xt", [S + PAD, B], BF16, isOutput=False)
    wts = nc.declare_dram_parameter("wts", [128, 6 * NCHUNK], BF16, isOutput=False)
    out = nc.declare_dram_parameter("out", [2, B], FP32, isOutput=True)

    with (
        nc.sbuf_tensor("w", [128, 6 * NCHUNK], BF16) as w,
        nc.sbuf_tensor("t0", [128, NCHUNK * B], BF16) as t0,
        nc.sbuf_tensor("t1", [128, NCHUNK * B], BF16) as t1,
        nc.sbuf_tensor("t64", [128, NCHUNK * B], BF16) as t64,
        nc.sbuf_tensor("q1", [128, NCHUNK * B], BF16) as q1,
        nc.sbuf_tensor("q64", [128, NCHUNK * B], BF16) as q64,
        nc.sbuf_tensor("res", [2, B], FP32) as res,
        nc.psum_tensor("acc", [2, B], FP32) as acc,
        nc.semaphore("wsem") as wsem,
        nc.semaphore("osem") as osem,
        nc.semaphore("vsem") as vsem,
        nc.semaphore("psem") as psem,
        nc.semaphore("st0") as st0,
        nc.semaphore("st1") as st1,
        nc.semaphore("st64") as st64,
        nc.Block() as block,
    ):
        # Each DMA gets its own semaphore: concurrent DMAs interleave
        # their 16 per-SDMA-engine sub-increments, so only a semaphore's
        # final total is a race-free wait value. Standalone wait_ge
        # instructions keep everything at walrus's one-sync-wait limit.
        #
        # Each x-shift is ONE fat strided DMA (partition p <- xt rows
        # {shift + p + 128k}): the cost model charges ~650ns of sequencer
        # issue per dma_start, so few fat DMAs beat many small ones. The
        # three loads are split across the two HWDGE queues (SP + ACT) to
        # overlap issue latency.
        def ch(tt, k):
            return tt[:, k * B : (k + 1) * B]

        def load(eng, tile, s, sem, half=None):
            # half=None: all NCHUNK chunks in one DMA; half=0/1: the low/
            # high two chunks, so consumers can start after half the bytes.
            nk, k0 = (NCHUNK, 0) if half is None else (NCHUNK // 2, half * 2)
            eng.dma_start(
                out=tile[:, k0 * B : (k0 + nk) * B].rearrange(
                    "p (k b) -> p k b", k=nk
                ),
                in_=xt[s + k0 * 128 : s + (k0 + nk) * 128].rearrange(
                    "(k p) b -> p k b", p=128
                ),
            ).then_inc(sem, 16)

        # Queue/order choice: w first on the ACT queue (tiny, clears the
        # DMA engines fast), t0 first on the SP queue, then the t1/t64
        # halves interleaved across both queues. The DMA engines serialize
        # at ~350 GB/s in arrival order, so this puts the tensors on the
        # critical path (t1 gates all DVE work) earliest.
        @block.sync
        def _(sync):
            load(sync, t0, 0, st0)
            load(sync, t64, NG, st64, half=0)
            load(sync, t64, NG, st64, half=1)
            sync.wait_ge(vsem, 2 * NCHUNK + 1)
            sync.dma_start(out=out[:], in_=res[:]).then_inc(osem, 16)
            sync.wait_ge(osem, 16)

        @block.scalar
        def _(scalar):
            scalar.dma_start(out=w[:], in_=wts[:]).then_inc(wsem, 16)
            load(scalar, t1, 1, st1, half=0)
            load(scalar, t1, 1, st1, half=1)

        @block.vector
        def _(vector):
            vector.wait_ge(st0, 16)
            vector.wait_ge(st1, 16)
            for k in range(2):
                vector.tensor_mul(ch(q1, k), ch(t0, k), ch(t1, k)).then_inc(vsem, 1)
            vector.wait_ge(st64, 16)
            for k in range(2):
                vector.tensor_mul(ch(q64, k), ch(t0, k), ch(t64, k)).then_inc(
                    vsem, 1
                )
            vector.wait_ge(st1, 32)
            for k in range(2, NCHUNK):
                vector.tensor_mul(ch(q1, k), ch(t0, k), ch(t1, k)).then_inc(vsem, 1)
            vector.wait_ge(st64, 32)
            for k in range(2, NCHUNK):
                vector.tensor_mul(ch(q64, k), ch(t0, k), ch(t64, k)).then_inc(
                    vsem, 1
                )
            vector.wait_ge(psem, 3 * NCHUNK)
            vector.tensor_copy(out=res[:], in_=acc[:]).then_inc(vsem, 1)

        @block.tensor
        def _(tensor):
            tensor.wait_ge(wsem, 16)
            tensor.wait_ge(st0, 16)
            # u-term matmuls first (need only w+t0), then the product
            # matmuls in the exact order DVE emits them (vsem counts).
            n_mm = 0
            for k in range(NCHUNK):
                tensor.matmul(
                    acc[:],
                    w[:, 6 * k : 6 * k + 2],
                    ch(t0, k),
                    start=(n_mm == 0),
                    stop=False,
                ).then_inc(psem, 1)
                n_mm += 1
            prod_order = [("q1", 0), ("q1", 1), ("q64", 0), ("q64", 1),
                          ("q1", 2), ("q1", 3), ("q64", 2), ("q64", 3)]
            qt = {"q1": (q1, 2), "q64": (q64, 4)}
            for i, (name, k) in enumerate(prod_order):
                tile, woff = qt[name]
                tensor.wait_ge(vsem, i + 1)
                tensor.matmul(
                    acc[:],
                    w[:, 6 * k + woff : 6 * k + woff + 2],
                    ch(tile, k),
                    start=False,
                    stop=(i == len(prod_order) - 1),
                ).then_inc(psem, 1)
                n_mm += 1

    return nc


_NC_CACHE = None


def _get_nc():
    global _NC_CACHE
    if _NC_CACHE is None:
        _NC_CACHE = _build_bass()
    return _NC_CACHE


def _split_bf16(v):
    """fp32 vector -> (hi, lo) bf16 pair with hi+lo ~= v to ~2^-17 rel."""
    hi = v.astype(ml_dtypes.bfloat16)
    lo = (v - hi.astype(np.float32)).astype(ml_dtypes.bfloat16)
    return hi, lo


def _prep_inputs(x, unary, binary, mask):
    """Host-side shard prep: masked diagonals + padded transposed spins."""
    wr = np.zeros(N, np.float32)
    wd = np.zeros(N, np.float32)
    wr[: N - 1] = np.diagonal(binary, 1) * np.diagonal(mask, 1)
    wd[: N - NG] = np.diagonal(binary, NG) * np.diagonal(mask, NG)
    u = np.asarray(unary, np.float32)

    xt = np.zeros((N + PAD, B), ml_dtypes.bfloat16)
    xt[:N] = np.asarray(x, np.float32).T.astype(ml_dtypes.bfloat16)

    in_maps = []
    for c in range(NCORES):
        base = c * S
        w = np.empty((128, 6 * NCHUNK), ml_dtypes.bfloat16)
        for k in range(NCHUNK):
            rows = slice(base + k * 128, base + k * 128 + 128)
            for t, vec in enumerate((u, wr, wd)):
                hi, lo = _split_bf16(vec[rows])
                w[:, 6 * k + 2 * t] = hi
                w[:, 6 * k + 2 * t + 1] = lo
        in_maps.append(
            {"xt": np.ascontiguousarray(xt[base : base + S + PAD]), "wts": w}
        )
    return in_maps


def kernel(x, unary, binary, mask):
    nc = _get_nc()
    in_maps = _prep_inputs(x, unary, binary, mask)
    res = run_bass_kernel_spmd(nc, in_maps, list(range(NCORES))).results
    parts = np.stack([r["out"] for r in res])  # [8, 2, B]
    return parts.sum(axis=(0, 1), dtype=np.float64).astype(np.float32)



# revision 2
# speedup vs baseline: 1.0278x; 1.0278x over previous
"""Ising log-energy kernel for Trainium2 (8 NeuronCores).

Reference computation (B=512 samples, N=4096 spins on a 64x64 grid):
    e[b] = sum_i u[i]*x[b,i] + sum_{i<j} (binary*mask)[i,j]*x[b,i]*x[b,j]

The mask is the nearest-neighbor upper-triangular grid mask: the only
nonzeros of w = binary*mask sit on the +1 and +64 off-diagonals. So

    e[b] = sum_i x[b,i] * (wr[i]*x[b,i+1] + u[i] + wd[i]*x[b,i+64])

with wr/wd the masked diagonals of `binary`. That's O(B*N) work.

Distribution: tensor-parallel over sites. Core c owns sites
[c*512, c*512+512) for all 512 samples; partial energies are summed on
the host. On-device layout is site-major ([site, batch] = [partition,
free]), 4 chunks of 128 sites per core.

Per chunk k, with t0/t1/t64 = x rows shifted 0/+1/+64 (fp16/fp8 - exact
for +-1 spins) and fp32 per-partition weight columns wr/u/wd:

    DVE : m1 = t1*wr + u          (tensor_scalar, fp32 scalars, fp16 out)
    ACT : b  = t64*wd             (activation Copy with per-partition scale)
    DVE : s  = m1 + b             (tensor_tensor, chunk-paired)
    DVE : p  = t0 * s             (tensor_tensor, chunk-paired; exact sign flip)
    PE  : acc[1,512] += ones.T @ p  (all weights folded out of the matmul)

All weights enter in fp32; intermediates are fp16 (2^-11 rounding), so
no hi/lo splitting is needed anywhere.
"""

import os
from contextlib import ExitStack
import sys

import numpy as np

for _p in ("/opt/trn_rl_repo", "/root/.axon_site/_ro/trn_rl_repo"):
    if os.path.isdir(_p) and _p not in sys.path:
        sys.path.insert(0, _p)

import ml_dtypes

import concourse.bass as bass
import concourse.mybir as mybir
from concourse.bass_utils import run_bass_kernel_spmd


N = 4096          # total spins (64x64 grid)
NG = 64           # grid side (down-neighbor stride)
B = 512           # batch
NCORES = 8
S = N // NCORES   # sites per core = 512
PAD = NG + 1      # extra x rows needed for the +64/+1 shifts
NCHUNK = S // 128  # 128-site chunks per core = 4

FP32 = mybir.dt.float32
FP16 = mybir.dt.float16
FP8 = mybir.dt.float8e4

AOP = mybir.AluOpType
AFT = mybir.ActivationFunctionType

NP_FP16 = np.float16
NP_FP8 = ml_dtypes.float8_e4m3


def _build_bass():
    """Raw Bass (no Tile): the local walrus build only encodes ONE sync
    wait per instruction, so all waits are standalone wait_ge on counting
    semaphores. Each tensor's half-DMAs share one HWDGE ring (FIFO per
    ring) so cumulative semaphore values are race-free."""
    nc = bass.Bass()
    xt16 = nc.declare_dram_parameter("xt16", [S + PAD, B], FP16, isOutput=False)
    xt8 = nc.declare_dram_parameter("xt8", [S + PAD, B], FP8, isOutput=False)
    wts = nc.declare_dram_parameter("wts", [128, 3 * NCHUNK], FP32, isOutput=False)
    out = nc.declare_dram_parameter("out", [1, B], FP32, isOutput=True)

    with (
        nc.sbuf_tensor("w", [128, 3 * NCHUNK], FP32) as w,
        nc.sbuf_tensor("t0", [128, NCHUNK * B], FP16) as t0,
        nc.sbuf_tensor("t1", [128, NCHUNK * B], FP8) as t1,
        nc.sbuf_tensor("t64", [128, NCHUNK * B], FP8) as t64,
        nc.sbuf_tensor("m1", [128, NCHUNK * B], FP16) as m1,
        nc.sbuf_tensor("bb", [128, NCHUNK * B], FP16) as bb,
        nc.sbuf_tensor("ss", [128, NCHUNK * B], FP16) as ss,
        nc.sbuf_tensor("pp", [128, NCHUNK * B], FP16) as pp,
        nc.sbuf_tensor("ones", [128, 1], FP16) as ones,
        nc.sbuf_tensor("res", [1, B], FP32) as res,
        nc.psum_tensor("acc", [1, B], FP32) as acc,
        nc.semaphore("st0") as st0,
        nc.semaphore("st1") as st1,
        nc.semaphore("st64") as st64,
        nc.semaphore("swt") as swt,
        nc.semaphore("sb") as sb,
        nc.semaphore("sv") as sv,
        nc.semaphore("sp") as sp,
        nc.semaphore("sr") as sr,
        nc.semaphore("so") as so,
        nc.Block() as block,
    ):
        def ch(tt, k, n=1):
            return tt[:, k * B : (k + n) * B]

        def wcol(k, j):
            return w[:, 3 * k + j : 3 * k + j + 1]

        def load(eng, tile, src, s, sem, half):
            # half=0/1: the low/high two chunks of the core's 4.
            nk, k0 = NCHUNK // 2, half * 2
            eng.dma_start(
                out=tile[:, k0 * B : (k0 + nk) * B].rearrange(
                    "p (k b) -> p k b", k=nk
                ),
                in_=src[s + k0 * 128 : s + (k0 + nk) * 128].rearrange(
                    "(k p) b -> p k b", p=128
                ),
            ).then_inc(sem, 16)

        # sync ring (FIFO): t1 halves first (m1 is first DVE work), then
        # t0 halves (p consumes them last); out-store at the end.
        @block.sync
        def _(sync):
            load(sync, t1, xt8, 1, st1, 0)
            load(sync, t1, xt8, 1, st1, 1)
            load(sync, t0, xt16, 0, st0, 0)
            load(sync, t0, xt16, 0, st0, 1)
            sync.wait_ge(sr, 1)
            sync.dma_start(out=out[:], in_=res[:]).then_inc(so, 16)
            sync.wait_ge(so, 16)

        # scalar (ACT) ring: weights first (tiny), then t64 halves; the
        # ACT engine then computes b_k as its own DMAs land.
        @block.scalar
        def _(scalar):
            scalar.dma_start(out=w[:], in_=wts[:]).then_inc(swt, 16)
            load(scalar, t64, xt8, NG, st64, 0)
            load(scalar, t64, xt8, NG, st64, 1)
            scalar.wait_ge(st64, 16)
            for k in range(2):
                scalar.activation(
                    ch(bb, k), ch(t64, k), AFT.Copy, scale=wcol(k, 2)
                ).then_inc(sb, 1)
            scalar.wait_ge(st64, 32)
            for k in range(2, NCHUNK):
                scalar.activation(
                    ch(bb, k), ch(t64, k), AFT.Copy, scale=wcol(k, 2)
                ).then_inc(sb, 1)
            scalar.wait_ge(sp, 1)
            scalar.activation(res[:], acc[:], AFT.Copy).then_inc(sr, 1)

        @block.vector
        def _(vector):
            vector.memset(ones[:], 1.0)
            vector.wait_ge(swt, 16)
            vector.wait_ge(st1, 16)
            for k in range(2):
                vector.tensor_scalar(
                    ch(m1, k), ch(t1, k), wcol(k, 0), wcol(k, 1), AOP.mult, AOP.add
                )
            vector.wait_ge(st1, 32)
            for k in range(2, NCHUNK):
                vector.tensor_scalar(
                    ch(m1, k), ch(t1, k), wcol(k, 0), wcol(k, 1), AOP.mult, AOP.add
                )
            vector.wait_ge(sb, 2)
            vector.tensor_add(ch(ss, 0, 2), ch(m1, 0, 2), ch(bb, 0, 2))
            vector.wait_ge(st0, 16)
            vector.tensor_mul(ch(pp, 0, 2), ch(t0, 0, 2), ch(ss, 0, 2)).then_inc(
                sv, 1
            )
            vector.wait_ge(sb, 4)
            vector.tensor_add(ch(ss, 2, 2), ch(m1, 2, 2), ch(bb, 2, 2))
            vector.wait_ge(st0, 32)
            vector.tensor_mul(ch(pp, 2, 2), ch(t0, 2, 2), ch(ss, 2, 2)).then_inc(
                sv, 1
            )

        @block.tensor
        def _(tensor):
            tensor.wait_ge(sv, 1)
            tensor.matmul(acc[:], ones[:], ch(pp, 0), start=True, stop=False)
            tensor.matmul(acc[:], ones[:], ch(pp, 1), start=False, stop=False)
            tensor.wait_ge(sv, 2)
            tensor.matmul(acc[:], ones[:], ch(pp, 2), start=False, stop=False)
            tensor.matmul(acc[:], ones[:], ch(pp, 3), start=False, stop=True).then_inc(
                sp, 1
            )

    return nc


_NC_CACHE = None


def _get_nc():
    global _NC_CACHE
    if _NC_CACHE is None:
        _NC_CACHE = _build_bass()
    return _NC_CACHE


def _prep_inputs(x, unary, binary, mask):
    """Host-side shard prep: masked diagonals + padded transposed spins."""
    wr = np.zeros(N, np.float32)
    wd = np.zeros(N, np.float32)
    wr[: N - 1] = np.diagonal(binary, 1) * np.diagonal(mask, 1)
    wd[: N - NG] = np.diagonal(binary, NG) * np.diagonal(mask, NG)
    u = np.asarray(unary, np.float32)

    xt = np.zeros((N + PAD, B), np.float32)
    xt[:N] = np.asarray(x, np.float32).T
    xt16 = xt.astype(NP_FP16)
    xt8 = xt.astype(NP_FP8)

    in_maps = []
    for c in range(NCORES):
        base = c * S
        w = np.empty((128, 3 * NCHUNK), np.float32)
        for k in range(NCHUNK):
            rows = slice(base + k * 128, base + k * 128 + 128)
            w[:, 3 * k + 0] = wr[rows]
            w[:, 3 * k + 1] = u[rows]
            w[:, 3 * k + 2] = wd[rows]
        in_maps.append(
            {
                "xt16": np.ascontiguousarray(xt16[base : base + S + PAD]),
                "xt8": np.ascontiguousarray(xt8[base : base + S + PAD]),
                "wts": w,
            }
        )
    return in_maps


def kernel(x, unary, binary, mask):
    nc = _get_nc()
    in_maps = _prep_inputs(x, unary, binary, mask)
    res = run_bass_kernel_spmd(nc, in_maps, list(range(NCORES))).results
    parts = np.stack([r["out"] for r in res])  # [8, 1, B]
    return parts.sum(axis=(0, 1), dtype=np.float64).astype(np.float32)


# revision 7
# speedup vs baseline: 1.0342x; 1.0062x over previous
"""Ising log-energy kernel for Trainium2 (8 NeuronCores).

Reference computation (B=512 samples, N=4096 spins on a 64x64 grid):
    e[b] = sum_i u[i]*x[b,i] + sum_{i<j} (binary*mask)[i,j]*x[b,i]*x[b,j]

The mask is the nearest-neighbor upper-triangular grid mask: the only
nonzeros of w = binary*mask sit on the +1 and +64 off-diagonals. So

    e[b] = sum_i x[b,i] * (wr[i]*x[b,i+1] + u[i] + wd[i]*x[b,i+64])

with wr/wd the masked diagonals of `binary`. That's O(B*N) work.

Distribution: tensor-parallel over sites. Core c owns sites
[c*512, c*512+512) for all 512 samples; partial energies are summed on
the host. On-device layout is site-major ([site, batch] = [partition,
free]), 4 chunks of 128 sites per core.

Per chunk k, with t0/t1/t64 = x rows shifted 0/+1/+64 (fp16/fp8 - exact
for +-1 spins) and fp32 per-partition weight columns wr/u/wd:

    DVE : m1 = t1*wr + u          (tensor_scalar, fp32 scalars, fp16 out)
    ACT : b  = t64*wd             (activation Copy with per-partition scale)
    DVE : s  = m1 + b             (tensor_tensor, chunk-paired)
    DVE : p  = t0 * s             (tensor_tensor, chunk-paired; exact sign flip)
    PE  : acc[1,512] += ones.T @ p  (all weights folded out of the matmul)

All weights enter in fp32; intermediates are fp16 (2^-11 rounding), so
no hi/lo splitting is needed anywhere.
"""

import os
from contextlib import ExitStack
import sys

import numpy as np

for _p in ("/opt/trn_rl_repo", "/root/.axon_site/_ro/trn_rl_repo"):
    if os.path.isdir(_p) and _p not in sys.path:
        sys.path.insert(0, _p)

import ml_dtypes

import concourse.bass as bass
import concourse.mybir as mybir
from concourse.bass_utils import run_bass_kernel_spmd


N = 4096          # total spins (64x64 grid)
NG = 64           # grid side (down-neighbor stride)
B = 512           # batch
NCORES = 8
S = N // NCORES   # sites per core = 512
PAD = NG + 1      # extra x rows needed for the +64/+1 shifts
NCHUNK = S // 128  # 128-site chunks per core = 4

FP32 = mybir.dt.float32
FP16 = mybir.dt.float16
FP8 = mybir.dt.float8e4

AOP = mybir.AluOpType
AFT = mybir.ActivationFunctionType

NP_FP16 = np.float16
NP_FP8 = ml_dtypes.float8_e4m3


def _build_bass():
    """Raw Bass (no Tile): the local walrus build only encodes ONE sync
    wait per instruction, so all waits are standalone wait_ge on counting
    semaphores. Each tensor's half-DMAs share one HWDGE ring (FIFO per
    ring) so cumulative semaphore values are race-free."""
    nc = bass.Bass()
    xt16 = nc.declare_dram_parameter("xt16", [S + PAD, B], FP16, isOutput=False)
    xt8 = nc.declare_dram_parameter("xt8", [S + PAD, B], FP8, isOutput=False)
    wts = nc.declare_dram_parameter("wts", [128, 3 * NCHUNK], FP32, isOutput=False)
    out = nc.declare_dram_parameter("out", [1, B], FP16, isOutput=True)

    with ExitStack() as ctx:
        w = ctx.enter_context(nc.sbuf_tensor("w", [128, 3 * NCHUNK], FP32))
        t0 = ctx.enter_context(nc.sbuf_tensor("t0", [128, NCHUNK * B], FP16))
        t1 = ctx.enter_context(nc.sbuf_tensor("t1", [128, NCHUNK * B], FP8))
        t64 = ctx.enter_context(nc.sbuf_tensor("t64", [128, NCHUNK * B], FP8))
        m1 = ctx.enter_context(nc.sbuf_tensor("m1", [128, NCHUNK * B], FP16))
        bb = ctx.enter_context(nc.sbuf_tensor("bb", [128, NCHUNK * B], FP16))
        ss = ctx.enter_context(nc.sbuf_tensor("ss", [128, NCHUNK * B], FP16))
        pp = ctx.enter_context(nc.sbuf_tensor("pp", [128, NCHUNK * B], FP16))
        ones = ctx.enter_context(nc.sbuf_tensor("ones", [128, 1], FP16))
        scr = ctx.enter_context(nc.sbuf_tensor("scr", [1, 1], FP16))
        res = ctx.enter_context(nc.sbuf_tensor("res", [1, B], FP16))
        acc = ctx.enter_context(nc.psum_tensor("acc", [1, B], FP32))
        st0 = ctx.enter_context(nc.semaphore("st0"))
        st1 = ctx.enter_context(nc.semaphore("st1"))
        st64 = ctx.enter_context(nc.semaphore("st64"))
        swt = ctx.enter_context(nc.semaphore("swt"))
        sb = ctx.enter_context(nc.semaphore("sb"))
        sv = ctx.enter_context(nc.semaphore("sv"))
        sp = ctx.enter_context(nc.semaphore("sp"))
        sr = ctx.enter_context(nc.semaphore("sr"))
        so = ctx.enter_context(nc.semaphore("so"))
        block = ctx.enter_context(nc.Block())
        def ch(tt, k, n=1):
            return tt[:, k * B : (k + n) * B]

        def wcol(k, j):
            return w[:, 3 * k + j : 3 * k + j + 1]

        def load(eng, tile, src, s, sem, half):
            # half=0/1: the low/high two chunks of the core's 4.
            nk, k0 = NCHUNK // 2, half * 2
            eng.dma_start(
                out=tile[:, k0 * B : (k0 + nk) * B].rearrange(
                    "p (k b) -> p k b", k=nk
                ),
                in_=src[s + k0 * 128 : s + (k0 + nk) * 128].rearrange(
                    "(k p) b -> p k b", p=128
                ),
            ).then_inc(sem, 16)

        # sync ring (FIFO): t64 halves first (they gate the slow ACT b
        # ops), then t0 halves (p consumes them later); out-store last.
        @block.sync
        def _(sync):
            load(sync, t64, xt8, NG, st64, 0)
            load(sync, t64, xt8, NG, st64, 1)
            load(sync, t0, xt16, 0, st0, 0)
            load(sync, t0, xt16, 0, st0, 1)
            sync.wait_ge(sr, 1)
            sync.dma_start(out=out[:], in_=res[:]).then_inc(so, 16)
            sync.wait_ge(so, 16)

        # pool (SWDGE) ring: weights + t1 halves, keeping the ACT stream
        # free of DMA-issue work so b ops start the moment t64 lands.
        @block.gpsimd
        def _(gpsimd):
            gpsimd.dma_start(out=w[:], in_=wts[:]).then_inc(swt, 16)
            load(gpsimd, t1, xt8, 1, st1, 0)
            load(gpsimd, t1, xt8, 1, st1, 1)

        # ACT: a dummy activation up front forces the one-time
        # ACT_TABLE_LOAD while DMAs are still in flight.
        @block.scalar
        def _(scalar):
            scalar.activation(scr[:], ones[0:1, 0:1], AFT.Copy, scale=1.0)
            scalar.wait_ge(swt, 16)
            scalar.wait_ge(st64, 16)
            for k in range(2):
                scalar.activation(
                    ch(bb, k), ch(t64, k), AFT.Copy, scale=wcol(k, 2)
                ).then_inc(sb, 1)
            scalar.wait_ge(st64, 32)
            for k in range(2, NCHUNK):
                scalar.activation(
                    ch(bb, k), ch(t64, k), AFT.Copy, scale=wcol(k, 2)
                ).then_inc(sb, 1)

        @block.vector
        def _(vector):
            vector.memset(ones[:], 1.0)
            vector.wait_ge(swt, 16)
            vector.wait_ge(st1, 16)
            for k in range(2):
                vector.tensor_scalar(
                    ch(m1, k), ch(t1, k), wcol(k, 0), wcol(k, 1), AOP.mult, AOP.add
                )
            vector.wait_ge(st1, 32)
            for k in range(2, NCHUNK):
                vector.tensor_scalar(
                    ch(m1, k), ch(t1, k), wcol(k, 0), wcol(k, 1), AOP.mult, AOP.add
                )
            vector.wait_ge(sb, 2)
            vector.tensor_add(ch(ss, 0, 2), ch(m1, 0, 2), ch(bb, 0, 2))
            vector.wait_ge(st0, 16)
            vector.tensor_mul(ch(pp, 0, 2), ch(t0, 0, 2), ch(ss, 0, 2)).then_inc(
                sv, 1
            )
            vector.wait_ge(sb, 4)
            vector.tensor_add(ch(ss, 2, 2), ch(m1, 2, 2), ch(bb, 2, 2))
            vector.wait_ge(st0, 32)
            # split the tail: p2 then p3 singles, so the last PE matmul
            # starts one DVE op earlier.
            vector.tensor_mul(ch(pp, 2), ch(t0, 2), ch(ss, 2)).then_inc(sv, 1)
            vector.tensor_mul(ch(pp, 3), ch(t0, 3), ch(ss, 3)).then_inc(sv, 1)
            vector.wait_ge(sp, 1)
            vector.tensor_copy(out=res[:], in_=acc[:]).then_inc(sr, 1)

        @block.tensor
        def _(tensor):
            tensor.wait_ge(sv, 1)
            tensor.matmul(acc[:], ones[:], ch(pp, 0), start=True, stop=False)
            tensor.matmul(acc[:], ones[:], ch(pp, 1), start=False, stop=False)
            tensor.wait_ge(sv, 2)
            tensor.matmul(acc[:], ones[:], ch(pp, 2), start=False, stop=False)
            tensor.wait_ge(sv, 3)
            tensor.matmul(acc[:], ones[:], ch(pp, 3), start=False, stop=True).then_inc(
                sp, 1
            )

    return nc


_NC_CACHE = None


def _get_nc():
    global _NC_CACHE
    if _NC_CACHE is None:
        _NC_CACHE = _build_bass()
    return _NC_CACHE


def _prep_inputs(x, unary, binary, mask):
    """Host-side shard prep: masked diagonals + padded transposed spins."""
    wr = np.zeros(N, np.float32)
    wd = np.zeros(N, np.float32)
    wr[: N - 1] = np.diagonal(binary, 1) * np.diagonal(mask, 1)
    wd[: N - NG] = np.diagonal(binary, NG) * np.diagonal(mask, NG)
    u = np.asarray(unary, np.float32)

    xt = np.zeros((N + PAD, B), np.float32)
    xt[:N] = np.asarray(x, np.float32).T
    xt16 = xt.astype(NP_FP16)
    xt8 = xt.astype(NP_FP8)

    in_maps = []
    for c in range(NCORES):
        base = c * S
        w = np.empty((128, 3 * NCHUNK), np.float32)
        for k in range(NCHUNK):
            rows = slice(base + k * 128, base + k * 128 + 128)
            w[:, 3 * k + 0] = wr[rows]
            w[:, 3 * k + 1] = u[rows]
            w[:, 3 * k + 2] = wd[rows]
        in_maps.append(
            {
                "xt16": np.ascontiguousarray(xt16[base : base + S + PAD]),
                "xt8": np.ascontiguousarray(xt8[base : base + S + PAD]),
                "wts": w,
            }
        )
    return in_maps


def kernel(x, unary, binary, mask):
    nc = _get_nc()
    in_maps = _prep_inputs(x, unary, binary, mask)
    res = run_bass_kernel_spmd(nc, in_maps, list(range(NCORES))).results
    parts = np.stack([np.asarray(r["out"], np.float32) for r in res])  # [8,1,B]
    return parts.sum(axis=(0, 1), dtype=np.float64).astype(np.float32)


# revision 9
# speedup vs baseline: 1.0827x; 1.0469x over previous
"""Ising log-energy kernel for Trainium2 (8 NeuronCores).

Reference computation (B=512 samples, N=4096 spins on a 64x64 grid):
    e[b] = sum_i u[i]*x[b,i] + sum_{i<j} (binary*mask)[i,j]*x[b,i]*x[b,j]

The mask is the nearest-neighbor upper-triangular grid mask: the only
nonzeros of w = binary*mask sit on the +1 and +64 off-diagonals. So

    e[b] = sum_i x[b,i] * (wr[i]*x[b,i+1] + u[i] + wd[i]*x[b,i+64])

with wr/wd the masked diagonals of `binary`. That's O(B*N) work.

Distribution: tensor-parallel over sites. Core c owns sites
[c*512, c*512+512) for all 512 samples; partial energies are summed on
the host. On-device layout is site-major ([site, batch] = [partition,
free]), 4 chunks of 128 sites per core.

Per chunk k, with t0/t1/t64 = x rows shifted 0/+1/+64 (fp16/fp8 - exact
for +-1 spins) and fp32 per-partition weight columns wr/u/wd:

    DVE/Pool: m1 = t1*wr + u    (tensor_scalar, fp32 AP scalars, fp16 out)
    ACT     : b  = t64*wd       (activation Copy with per-partition scale)
    DVE     : s  = m1 + b       (tensor_tensor, chunk-paired)
    DVE     : p  = t0 * s       (tensor_tensor; exact sign flip)
    PE      : acc[1,512] += ones.T @ p_k   (weights folded out of matmul)

Host buffers are pre-chunked so every DMA is a contiguous [128, X] read
(the V2 strided gathers ran at ~45-85 GB/s; contiguous ~250+). t64 and
t1 chunks are interleaved in one fp8 buffer so each quarter-DMA
unblocks both the ACT b-op and the DVE m1-op for that chunk.
"""

import os
from contextlib import ExitStack
import sys

import numpy as np

for _p in ("/opt/trn_rl_repo", "/root/.axon_site/_ro/trn_rl_repo"):
    if os.path.isdir(_p) and _p not in sys.path:
        sys.path.insert(0, _p)

import ml_dtypes

import concourse.bass as bass
import concourse.mybir as mybir
from concourse.bass_utils import run_bass_kernel_spmd


N = 4096          # total spins (64x64 grid)
NG = 64           # grid side (down-neighbor stride)
B = 512           # batch
NCORES = 8
S = N // NCORES   # sites per core = 512
NCHUNK = S // 128  # 128-site chunks per core = 4

FP32 = mybir.dt.float32
FP16 = mybir.dt.float16
FP8 = mybir.dt.float8e4

AOP = mybir.AluOpType
AFT = mybir.ActivationFunctionType

NP_FP16 = np.float16
NP_FP8 = ml_dtypes.float8_e4m3

POOL_M1 = False  # compute m1 for chunks 2,3 on the Pool engine


def _build_bass():
    """Raw Bass (no Tile): the local walrus build only encodes ONE sync
    wait per instruction, so all waits are standalone wait_ge on counting
    semaphores. DMAs of one tensor share one ring (FIFO per ring) so
    cumulative semaphore values are race-free."""
    nc = bass.Bass()
    # bufA quarter k = [t64 chunk k | t1 chunk k], each [128, 512] fp8.
    bufA = nc.declare_dram_parameter("bufA", [128, 2 * NCHUNK * B], FP8, isOutput=False)
    bufT0 = nc.declare_dram_parameter("bufT0", [128, NCHUNK * B], FP16, isOutput=False)
    wts = nc.declare_dram_parameter("wts", [128, 3 * NCHUNK], FP32, isOutput=False)
    out = nc.declare_dram_parameter("out", [1, B], FP16, isOutput=True)

    with ExitStack() as ctx:
        w = ctx.enter_context(nc.sbuf_tensor("w", [128, 3 * NCHUNK], FP32))
        tA = ctx.enter_context(nc.sbuf_tensor("tA", [128, 2 * NCHUNK * B], FP8))
        t0 = ctx.enter_context(nc.sbuf_tensor("t0", [128, NCHUNK * B], FP16))
        m1 = ctx.enter_context(nc.sbuf_tensor("m1", [128, NCHUNK * B], FP16))
        bb = ctx.enter_context(nc.sbuf_tensor("bb", [128, NCHUNK * B], FP16))
        ss = ctx.enter_context(nc.sbuf_tensor("ss", [128, NCHUNK * B], FP16))
        pp = ctx.enter_context(nc.sbuf_tensor("pp", [128, NCHUNK * B], FP16))
        ones = ctx.enter_context(nc.sbuf_tensor("ones", [128, 1], FP16))
        scr = ctx.enter_context(nc.sbuf_tensor("scr", [1, 1], FP16))
        res = ctx.enter_context(nc.sbuf_tensor("res", [1, B], FP16))
        acc = ctx.enter_context(nc.psum_tensor("acc", [1, B], FP32))
        sA = ctx.enter_context(nc.semaphore("sA"))
        st0 = ctx.enter_context(nc.semaphore("st0"))
        swt = ctx.enter_context(nc.semaphore("swt"))
        sb = ctx.enter_context(nc.semaphore("sb"))
        sm = ctx.enter_context(nc.semaphore("sm"))
        sv = ctx.enter_context(nc.semaphore("sv"))
        sp = ctx.enter_context(nc.semaphore("sp"))
        sr = ctx.enter_context(nc.semaphore("sr"))
        so = ctx.enter_context(nc.semaphore("so"))
        block = ctx.enter_context(nc.Block())

        def t64c(k):
            return tA[:, 2 * k * B : (2 * k + 1) * B]

        def t1c(k):
            return tA[:, (2 * k + 1) * B : (2 * k + 2) * B]

        def ch(tt, k, n=1):
            return tt[:, k * B : (k + n) * B]

        def wcol(k, j):
            return w[:, 3 * k + j : 3 * k + j + 1]

        # sync ring (FIFO): the four [t64|t1] quarters, then out-store.
        @block.sync
        def _(sync):
            for q in range(NCHUNK):
                sync.dma_start(
                    out=tA[:, q * 2 * B : (q + 1) * 2 * B],
                    in_=bufA[:, q * 2 * B : (q + 1) * 2 * B],
                ).then_inc(sA, 16)
            sync.wait_ge(sr, 1)
            sync.dma_start(out=out[:], in_=res[:]).then_inc(so, 16)
            sync.wait_ge(so, 16)

        # ACT: dummy activation first (forces the one-time ACT_TABLE_LOAD
        # while DMAs fly), t0 halves on this ring, then the b ops.
        @block.scalar
        def _(scalar):
            scalar.activation(scr[:], ones[0:1, 0:1], AFT.Copy, scale=1.0)
            for h in range(2):
                scalar.dma_start(
                    out=t0[:, h * 2 * B : (h + 1) * 2 * B],
                    in_=bufT0[:, h * 2 * B : (h + 1) * 2 * B],
                ).then_inc(st0, 16)
            scalar.wait_ge(swt, 16)
            for k in range(NCHUNK):
                scalar.wait_ge(sA, 16 * (k + 1))
                scalar.activation(
                    ch(bb, k), t64c(k), AFT.Copy, scale=wcol(k, 2)
                ).then_inc(sb, 1)
            scalar.wait_ge(sp, 1)
            scalar.activation(res[:], acc[:], AFT.Copy).then_inc(sr, 1)

        # pool: weights (tiny, SWDGE) + m1 for chunks 2,3.
        @block.gpsimd
        def _(gpsimd):
            gpsimd.dma_start(out=w[:], in_=wts[:]).then_inc(swt, 16)
            if POOL_M1:
                gpsimd.wait_ge(swt, 16)
                for k in (2, 3):
                    gpsimd.wait_ge(sA, 16 * (k + 1))
                    gpsimd.tensor_scalar(
                        ch(m1, k), t1c(k), wcol(k, 0), wcol(k, 1),
                        AOP.mult, AOP.add,
                    ).then_inc(sm, 1)

        @block.vector
        def _(vector):
            vector.memset(ones[:], 1.0)
            vector.wait_ge(swt, 16)
            for k in range(2):
                vector.wait_ge(sA, 16 * (k + 1))
                vector.tensor_scalar(
                    ch(m1, k), t1c(k), wcol(k, 0), wcol(k, 1), AOP.mult, AOP.add
                )
            if not POOL_M1:
                for k in (2, 3):
                    vector.wait_ge(sA, 16 * (k + 1))
                    vector.tensor_scalar(
                        ch(m1, k), t1c(k), wcol(k, 0), wcol(k, 1),
                        AOP.mult, AOP.add,
                    )
            vector.wait_ge(sb, 2)
            vector.tensor_add(ch(ss, 0, 2), ch(m1, 0, 2), ch(bb, 0, 2))
            vector.wait_ge(st0, 16)
            vector.tensor_mul(ch(pp, 0, 2), ch(t0, 0, 2), ch(ss, 0, 2)).then_inc(
                sv, 1
            )
            vector.wait_ge(sb, 4)
            if POOL_M1:
                vector.wait_ge(sm, 2)
            vector.tensor_add(ch(ss, 2, 2), ch(m1, 2, 2), ch(bb, 2, 2))
            vector.wait_ge(st0, 32)
            # split the tail so the last PE matmul starts one op earlier
            vector.tensor_mul(ch(pp, 2), ch(t0, 2), ch(ss, 2)).then_inc(sv, 1)
            vector.tensor_mul(ch(pp, 3), ch(t0, 3), ch(ss, 3)).then_inc(sv, 1)

        @block.tensor
        def _(tensor):
            tensor.wait_ge(sv, 1)
            tensor.matmul(acc[:], ones[:], ch(pp, 0), start=True, stop=False)
            tensor.matmul(acc[:], ones[:], ch(pp, 1), start=False, stop=False)
            tensor.wait_ge(sv, 2)
            tensor.matmul(acc[:], ones[:], ch(pp, 2), start=False, stop=False)
            tensor.wait_ge(sv, 3)
            tensor.matmul(acc[:], ones[:], ch(pp, 3), start=False, stop=True).then_inc(
                sp, 1
            )

    return nc


_NC_CACHE = None


def _get_nc():
    global _NC_CACHE
    if _NC_CACHE is None:
        _NC_CACHE = _build_bass()
    return _NC_CACHE


def _prep_inputs(x, unary, binary, mask):
    """Host-side shard prep: masked diagonals + pre-chunked spin tiles."""
    wr = np.zeros(N, np.float32)
    wd = np.zeros(N, np.float32)
    wr[: N - 1] = np.diagonal(binary, 1) * np.diagonal(mask, 1)
    wd[: N - NG] = np.diagonal(binary, NG) * np.diagonal(mask, NG)
    u = np.asarray(unary, np.float32)

    PADROWS = N + NG + 1
    xt = np.zeros((PADROWS, B), np.float32)
    xt[:N] = np.asarray(x, np.float32).T
    xt16 = xt.astype(NP_FP16)
    xt8 = xt.astype(NP_FP8)

    in_maps = []
    for c in range(NCORES):
        base = c * S
        w = np.empty((128, 3 * NCHUNK), np.float32)
        bufA = np.empty((128, 2 * NCHUNK, B), NP_FP8)
        bufT0 = np.empty((128, NCHUNK, B), NP_FP16)
        for k in range(NCHUNK):
            r0 = base + k * 128
            w[:, 3 * k + 0] = wr[r0 : r0 + 128]
            w[:, 3 * k + 1] = u[r0 : r0 + 128]
            w[:, 3 * k + 2] = wd[r0 : r0 + 128]
            bufA[:, 2 * k] = xt8[r0 + NG : r0 + NG + 128]   # t64 chunk k
            bufA[:, 2 * k + 1] = xt8[r0 + 1 : r0 + 1 + 128]  # t1 chunk k
            bufT0[:, k] = xt16[r0 : r0 + 128]                # t0 chunk k
        in_maps.append(
            {
                "bufA": bufA.reshape(128, 2 * NCHUNK * B),
                "bufT0": bufT0.reshape(128, NCHUNK * B),
                "wts": w,
            }
        )
    return in_maps


def kernel(x, unary, binary, mask):
    nc = _get_nc()
    in_maps = _prep_inputs(x, unary, binary, mask)
    res = run_bass_kernel_spmd(nc, in_maps, list(range(NCORES))).results
    parts = np.stack([np.asarray(r["out"], np.float32) for r in res])  # [8,1,B]
    return parts.sum(axis=(0, 1), dtype=np.float64).astype(np.float32)


# revision 10
# speedup vs baseline: 1.1386x; 1.0516x over previous
"""Ising log-energy kernel for Trainium2 (8 NeuronCores).

Reference computation (B=512 samples, N=4096 spins on a 64x64 grid):
    e[b] = sum_i u[i]*x[b,i] + sum_{i<j} (binary*mask)[i,j]*x[b,i]*x[b,j]

The mask is the nearest-neighbor upper-triangular grid mask: the only
nonzeros of w = binary*mask sit on the +1 and +64 off-diagonals. So

    e[b] = sum_i x[b,i] * (wr[i]*x[b,i+1] + u[i] + wd[i]*x[b,i+64])

with wr/wd the masked diagonals of `binary`. That's O(B*N) work.

Distribution: tensor-parallel over sites. Core c owns sites
[c*512, c*512+512) for all 512 samples; partial energies are summed on
the host. On-device layout is site-major ([site, batch] = [partition,
free]), 4 chunks of 128 sites per core.

Per chunk k, with t0/t1/t64 = x rows shifted 0/+1/+64 (fp16/fp8 - exact
for +-1 spins) and fp32 per-partition weight columns wr/u/wd:

    DVE/Pool: m1 = t1*wr + u    (tensor_scalar, fp32 AP scalars, fp16 out)
    ACT     : b  = t64*wd       (activation Copy with per-partition scale)
    DVE     : s  = m1 + b       (tensor_tensor, chunk-paired)
    DVE     : p  = t0 * s       (tensor_tensor; exact sign flip)
    PE      : acc[1,512] += ones.T @ p_k   (weights folded out of matmul)

Host buffers are pre-chunked so every DMA is a contiguous [128, X] read
(the V2 strided gathers ran at ~45-85 GB/s; contiguous ~250+). t64 and
t1 chunks are interleaved in one fp8 buffer so each quarter-DMA
unblocks both the ACT b-op and the DVE m1-op for that chunk.
"""

import os
from contextlib import ExitStack
import sys

import numpy as np

for _p in ("/opt/trn_rl_repo", "/root/.axon_site/_ro/trn_rl_repo"):
    if os.path.isdir(_p) and _p not in sys.path:
        sys.path.insert(0, _p)

import ml_dtypes

import concourse.bass as bass
import concourse.mybir as mybir
from concourse.bass_utils import run_bass_kernel_spmd


N = 4096          # total spins (64x64 grid)
NG = 64           # grid side (down-neighbor stride)
B = 512           # batch
NCORES = 8
S = N // NCORES   # sites per core = 512
NCHUNK = S // 128  # 128-site chunks per core = 4

FP32 = mybir.dt.float32
FP16 = mybir.dt.float16
FP8 = mybir.dt.float8e4

AOP = mybir.AluOpType
AFT = mybir.ActivationFunctionType

NP_FP16 = np.float16
NP_FP8 = ml_dtypes.float8_e4m3

POOL_M1 = False  # compute m1 for chunks 2,3 on the Pool engine


def _build_bass():
    """Raw Bass (no Tile): the local walrus build only encodes ONE sync
    wait per instruction, so all waits are standalone wait_ge on counting
    semaphores. DMAs of one tensor share one ring (FIFO per ring) so
    cumulative semaphore values are race-free."""
    nc = bass.Bass()
    # bufA quarter k = [t64 chunk k | t1 chunk k], each [128, 512] fp8.
    bufA = nc.declare_dram_parameter("bufA", [128, 2 * NCHUNK * B], FP8, isOutput=False)
    bufT0 = nc.declare_dram_parameter("bufT0", [128, NCHUNK * B], FP16, isOutput=False)
    wts = nc.declare_dram_parameter("wts", [128, 3 * NCHUNK], FP32, isOutput=False)
    out = nc.declare_dram_parameter("out", [1, B], FP16, isOutput=True)

    with ExitStack() as ctx:
        w = ctx.enter_context(nc.sbuf_tensor("w", [128, 3 * NCHUNK], FP32))
        tA = ctx.enter_context(nc.sbuf_tensor("tA", [128, 2 * NCHUNK * B], FP8))
        t0 = ctx.enter_context(nc.sbuf_tensor("t0", [128, NCHUNK * B], FP16))
        m1 = ctx.enter_context(nc.sbuf_tensor("m1", [128, NCHUNK * B], FP16))
        bb = ctx.enter_context(nc.sbuf_tensor("bb", [128, NCHUNK * B], FP16))
        ss = ctx.enter_context(nc.sbuf_tensor("ss", [128, NCHUNK * B], FP16))
        pp = ctx.enter_context(nc.sbuf_tensor("pp", [128, NCHUNK * B], FP16))
        ones = ctx.enter_context(nc.sbuf_tensor("ones", [128, 1], FP16))
        scr = ctx.enter_context(nc.sbuf_tensor("scr", [1, 1], FP16))
        res = ctx.enter_context(nc.sbuf_tensor("res", [1, B], FP16))
        acc = ctx.enter_context(nc.psum_tensor("acc", [1, B], FP32))
        sA = ctx.enter_context(nc.semaphore("sA"))
        st0 = ctx.enter_context(nc.semaphore("st0"))
        swt = ctx.enter_context(nc.semaphore("swt"))
        sb = ctx.enter_context(nc.semaphore("sb"))
        sm = ctx.enter_context(nc.semaphore("sm"))
        sv = ctx.enter_context(nc.semaphore("sv"))
        sp = ctx.enter_context(nc.semaphore("sp"))
        sr = ctx.enter_context(nc.semaphore("sr"))
        so = ctx.enter_context(nc.semaphore("so"))
        block = ctx.enter_context(nc.Block())

        def t64c(k):
            return tA[:, 2 * k * B : (2 * k + 1) * B]

        def t1c(k):
            return tA[:, (2 * k + 1) * B : (2 * k + 2) * B]

        def ch(tt, k, n=1):
            return tt[:, k * B : (k + n) * B]

        def wcol(k, j):
            return w[:, 3 * k + j : 3 * k + j + 1]

        # sync ring (FIFO): ALL x-traffic in priority order - the four
        # [t64|t1] quarters (gate DVE+ACT chunk k), then t0 halves
        # (needed later, by p), then out-store. One ring avoids the
        # packet-granularity round-robin between rings that delayed the
        # critical first quarter in V3.
        @block.sync
        def _(sync):
            for q in range(NCHUNK):
                sync.dma_start(
                    out=tA[:, q * 2 * B : (q + 1) * 2 * B],
                    in_=bufA[:, q * 2 * B : (q + 1) * 2 * B],
                ).then_inc(sA, 16)
            for h in range(2):
                sync.dma_start(
                    out=t0[:, h * 2 * B : (h + 1) * 2 * B],
                    in_=bufT0[:, h * 2 * B : (h + 1) * 2 * B],
                ).then_inc(st0, 16)
            sync.wait_ge(sr, 1)
            sync.dma_start(out=out[:], in_=res[:]).then_inc(so, 16)
            sync.wait_ge(so, 16)

        # ACT: dummy activation first (forces the one-time ACT_TABLE_LOAD
        # while DMAs fly), then the b ops; no DMA-issue work here.
        @block.scalar
        def _(scalar):
            scalar.activation(scr[:], ones[0:1, 0:1], AFT.Copy, scale=1.0)
            scalar.wait_ge(swt, 16)
            for k in range(NCHUNK):
                scalar.wait_ge(sA, 16 * (k + 1))
                scalar.activation(
                    ch(bb, k), t64c(k), AFT.Copy, scale=wcol(k, 2)
                ).then_inc(sb, 1)
            scalar.wait_ge(sp, 1)
            scalar.activation(res[:], acc[:], AFT.Copy).then_inc(sr, 1)

        # pool: weights (tiny, SWDGE) + m1 for chunks 2,3.
        @block.gpsimd
        def _(gpsimd):
            gpsimd.dma_start(out=w[:], in_=wts[:]).then_inc(swt, 16)
            if POOL_M1:
                gpsimd.wait_ge(swt, 16)
                for k in (2, 3):
                    gpsimd.wait_ge(sA, 16 * (k + 1))
                    gpsimd.tensor_scalar(
                        ch(m1, k), t1c(k), wcol(k, 0), wcol(k, 1),
                        AOP.mult, AOP.add,
                    ).then_inc(sm, 1)

        @block.vector
        def _(vector):
            vector.memset(ones[:], 1.0)
            vector.wait_ge(swt, 16)
            for k in range(2):
                vector.wait_ge(sA, 16 * (k + 1))
                vector.tensor_scalar(
                    ch(m1, k), t1c(k), wcol(k, 0), wcol(k, 1), AOP.mult, AOP.add
                )
            if not POOL_M1:
                for k in (2, 3):
                    vector.wait_ge(sA, 16 * (k + 1))
                    vector.tensor_scalar(
                        ch(m1, k), t1c(k), wcol(k, 0), wcol(k, 1),
                        AOP.mult, AOP.add,
                    )
            vector.wait_ge(sb, 2)
            vector.tensor_add(ch(ss, 0, 2), ch(m1, 0, 2), ch(bb, 0, 2))
            vector.wait_ge(st0, 16)
            vector.tensor_mul(ch(pp, 0, 2), ch(t0, 0, 2), ch(ss, 0, 2)).then_inc(
                sv, 1
            )
            vector.wait_ge(sb, 4)
            if POOL_M1:
                vector.wait_ge(sm, 2)
            vector.tensor_add(ch(ss, 2, 2), ch(m1, 2, 2), ch(bb, 2, 2))
            vector.wait_ge(st0, 32)
            # split the tail so the last PE matmul starts one op earlier
            vector.tensor_mul(ch(pp, 2), ch(t0, 2), ch(ss, 2)).then_inc(sv, 1)
            vector.tensor_mul(ch(pp, 3), ch(t0, 3), ch(ss, 3)).then_inc(sv, 1)

        @block.tensor
        def _(tensor):
            tensor.wait_ge(sv, 1)
            tensor.matmul(acc[:], ones[:], ch(pp, 0), start=True, stop=False)
            tensor.matmul(acc[:], ones[:], ch(pp, 1), start=False, stop=False)
            tensor.wait_ge(sv, 2)
            tensor.matmul(acc[:], ones[:], ch(pp, 2), start=False, stop=False)
            tensor.wait_ge(sv, 3)
            tensor.matmul(acc[:], ones[:], ch(pp, 3), start=False, stop=True).then_inc(
                sp, 1
            )

    return nc


_NC_CACHE = None


def _get_nc():
    global _NC_CACHE
    if _NC_CACHE is None:
        _NC_CACHE = _build_bass()
    return _NC_CACHE


def _prep_inputs(x, unary, binary, mask):
    """Host-side shard prep: masked diagonals + pre-chunked spin tiles."""
    wr = np.zeros(N, np.float32)
    wd = np.zeros(N, np.float32)
    wr[: N - 1] = np.diagonal(binary, 1) * np.diagonal(mask, 1)
    wd[: N - NG] = np.diagonal(binary, NG) * np.diagonal(mask, NG)
    u = np.asarray(unary, np.float32)

    PADROWS = N + NG + 1
    xt = np.zeros((PADROWS, B), np.float32)
    xt[:N] = np.asarray(x, np.float32).T
    xt16 = xt.astype(NP_FP16)
    xt8 = xt.astype(NP_FP8)

    in_maps = []
    for c in range(NCORES):
        base = c * S
        w = np.empty((128, 3 * NCHUNK), np.float32)
        bufA = np.empty((128, 2 * NCHUNK, B), NP_FP8)
        bufT0 = np.empty((128, NCHUNK, B), NP_FP16)
        for k in range(NCHUNK):
            r0 = base + k * 128
            w[:, 3 * k + 0] = wr[r0 : r0 + 128]
            w[:, 3 * k + 1] = u[r0 : r0 + 128]
            w[:, 3 * k + 2] = wd[r0 : r0 + 128]
            bufA[:, 2 * k] = xt8[r0 + NG : r0 + NG + 128]   # t64 chunk k
            bufA[:, 2 * k + 1] = xt8[r0 + 1 : r0 + 1 + 128]  # t1 chunk k
            bufT0[:, k] = xt16[r0 : r0 + 128]                # t0 chunk k
        in_maps.append(
            {
                "bufA": bufA.reshape(128, 2 * NCHUNK * B),
                "bufT0": bufT0.reshape(128, NCHUNK * B),
                "wts": w,
            }
        )
    return in_maps


def kernel(x, unary, binary, mask):
    nc = _get_nc()
    in_maps = _prep_inputs(x, unary, binary, mask)
    res = run_bass_kernel_spmd(nc, in_maps, list(range(NCORES))).results
    parts = np.stack([np.asarray(r["out"], np.float32) for r in res])  # [8,1,B]
    return parts.sum(axis=(0, 1), dtype=np.float64).astype(np.float32)


# revision 12
# speedup vs baseline: 1.1419x; 1.0029x over previous
"""Ising log-energy kernel for Trainium2 (8 NeuronCores).

Reference computation (B=512 samples, N=4096 spins on a 64x64 grid):
    e[b] = sum_i u[i]*x[b,i] + sum_{i<j} (binary*mask)[i,j]*x[b,i]*x[b,j]

The mask is the nearest-neighbor upper-triangular grid mask: the only
nonzeros of w = binary*mask sit on the +1 and +64 off-diagonals. So

    e[b] = sum_i x[b,i] * (wr[i]*x[b,i+1] + u[i] + wd[i]*x[b,i+64])

with wr/wd the masked diagonals of `binary`. That's O(B*N) work.

Distribution: tensor-parallel over sites. Core c owns sites
[c*512, c*512+512) for all 512 samples; partial energies are summed on
the host. On-device layout is site-major ([site, batch] = [partition,
free]), 4 chunks of 128 sites per core.

Per chunk k, with t0/t1/t64 = x rows shifted 0/+1/+64 (fp16/fp8 - exact
for +-1 spins) and fp32 per-partition weight columns wr/u/wd:

    DVE/Pool: m1 = t1*wr + u    (tensor_scalar, fp32 AP scalars, fp16 out)
    ACT     : b  = t64*wd       (activation Copy with per-partition scale)
    DVE     : s  = m1 + b       (tensor_tensor, chunk-paired)
    DVE     : p  = t0 * s       (tensor_tensor; exact sign flip)
    PE      : acc[1,512] += ones.T @ p_k   (weights folded out of matmul)

Host buffers are pre-chunked so every DMA is a contiguous [128, X] read
(the V2 strided gathers ran at ~45-85 GB/s; contiguous ~250+). t64 and
t1 chunks are interleaved in one fp8 buffer so each quarter-DMA
unblocks both the ACT b-op and the DVE m1-op for that chunk.
"""

import os
from contextlib import ExitStack
import sys

import numpy as np

for _p in ("/opt/trn_rl_repo", "/root/.axon_site/_ro/trn_rl_repo"):
    if os.path.isdir(_p) and _p not in sys.path:
        sys.path.insert(0, _p)

import ml_dtypes

import concourse.bass as bass
import concourse.mybir as mybir
from concourse.bass_utils import run_bass_kernel_spmd


N = 4096          # total spins (64x64 grid)
NG = 64           # grid side (down-neighbor stride)
B = 512           # batch
NCORES = 8
S = N // NCORES   # sites per core = 512
NCHUNK = S // 128  # 128-site chunks per core = 4

FP32 = mybir.dt.float32
FP16 = mybir.dt.float16
FP8 = mybir.dt.float8e4

AOP = mybir.AluOpType
AFT = mybir.ActivationFunctionType

NP_FP16 = np.float16
NP_FP8 = ml_dtypes.float8_e4m3

POOL_M1 = False  # compute m1 for chunks 2,3 on the Pool engine


def _build_bass():
    """Raw Bass (no Tile): the local walrus build only encodes ONE sync
    wait per instruction, so all waits are standalone wait_ge on counting
    semaphores. DMAs of one tensor share one ring (FIFO per ring) so
    cumulative semaphore values are race-free."""
    nc = bass.Bass()
    # bufA quarter k = [t64 chunk k | t1 chunk k], each [128, 512] fp8.
    bufA = nc.declare_dram_parameter("bufA", [128, 2 * NCHUNK * B], FP8, isOutput=False)
    bufT0 = nc.declare_dram_parameter("bufT0", [128, NCHUNK * B], FP16, isOutput=False)
    wts = nc.declare_dram_parameter("wts", [128, 3 * NCHUNK], FP32, isOutput=False)
    out = nc.declare_dram_parameter("out", [1, B], FP16, isOutput=True)

    with ExitStack() as ctx:
        w = ctx.enter_context(nc.sbuf_tensor("w", [128, 3 * NCHUNK], FP32))
        tA = ctx.enter_context(nc.sbuf_tensor("tA", [128, 2 * NCHUNK * B], FP8))
        t0 = ctx.enter_context(nc.sbuf_tensor("t0", [128, NCHUNK * B], FP16))
        m1 = ctx.enter_context(nc.sbuf_tensor("m1", [128, NCHUNK * B], FP16))
        bb = ctx.enter_context(nc.sbuf_tensor("bb", [128, NCHUNK * B], FP16))
        ss = ctx.enter_context(nc.sbuf_tensor("ss", [128, NCHUNK * B], FP16))
        pp = ctx.enter_context(nc.sbuf_tensor("pp", [128, NCHUNK * B], FP16))
        ones = ctx.enter_context(nc.sbuf_tensor("ones", [128, 1], FP16))
        scr = ctx.enter_context(nc.sbuf_tensor("scr", [1, 1], FP16))
        res = ctx.enter_context(nc.sbuf_tensor("res", [1, B], FP16))
        acc = ctx.enter_context(nc.psum_tensor("acc", [1, B], FP32))
        # One semaphore per waited DMA: the 16 SDMA engines' sub-increments
        # interleave across concurrent DMAs, so intermediate cumulative
        # values of a shared semaphore are NOT race-free.
        sA = [
            ctx.enter_context(nc.semaphore(f"sA{q}")) for q in range(NCHUNK)
        ]
        st0 = [ctx.enter_context(nc.semaphore(f"st0{h}")) for h in range(2)]
        swt = ctx.enter_context(nc.semaphore("swt"))
        sb = ctx.enter_context(nc.semaphore("sb"))
        sm = ctx.enter_context(nc.semaphore("sm"))
        sv = ctx.enter_context(nc.semaphore("sv"))
        sp = ctx.enter_context(nc.semaphore("sp"))
        sr = ctx.enter_context(nc.semaphore("sr"))
        so = ctx.enter_context(nc.semaphore("so"))
        block = ctx.enter_context(nc.Block())

        def t64c(k):
            return tA[:, 2 * k * B : (2 * k + 1) * B]

        def t1c(k):
            return tA[:, (2 * k + 1) * B : (2 * k + 2) * B]

        def ch(tt, k, n=1):
            return tt[:, k * B : (k + n) * B]

        def wcol(k, j):
            return w[:, 3 * k + j : 3 * k + j + 1]

        # sync ring (FIFO): ALL x-traffic in priority order - the four
        # [t64|t1] quarters (gate DVE+ACT chunk k), then t0 halves
        # (needed later, by p), then out-store. One ring avoids the
        # packet-granularity round-robin between rings that delayed the
        # critical first quarter in V3.
        @block.sync
        def _(sync):
            for q in range(NCHUNK):
                sync.dma_start(
                    out=tA[:, q * 2 * B : (q + 1) * 2 * B],
                    in_=bufA[:, q * 2 * B : (q + 1) * 2 * B],
                ).then_inc(sA[q], 16)
            for h in range(2):
                sync.dma_start(
                    out=t0[:, h * 2 * B : (h + 1) * 2 * B],
                    in_=bufT0[:, h * 2 * B : (h + 1) * 2 * B],
                ).then_inc(st0[h], 16)
            sync.wait_ge(sr, 1)
            sync.dma_start(out=out[:], in_=res[:]).then_inc(so, 16)
            sync.wait_ge(so, 16)

        # ACT: dummy activation first (forces the one-time ACT_TABLE_LOAD
        # while DMAs fly), then the b ops; no DMA-issue work here.
        @block.scalar
        def _(scalar):
            scalar.activation(scr[:], ones[0:1, 0:1], AFT.Copy, scale=1.0)
            scalar.wait_ge(swt, 16)
            for k in range(NCHUNK):
                scalar.wait_ge(sA[k], 16)
                scalar.activation(
                    ch(bb, k), t64c(k), AFT.Copy, scale=wcol(k, 2)
                ).then_inc(sb, 1)
            scalar.wait_ge(sp, 1)
            scalar.activation(res[:], acc[:], AFT.Copy).then_inc(sr, 1)

        # pool: weights (tiny, SWDGE) + m1 for chunks 2,3.
        @block.gpsimd
        def _(gpsimd):
            gpsimd.dma_start(out=w[:], in_=wts[:]).then_inc(swt, 16)
            if POOL_M1:
                gpsimd.wait_ge(swt, 16)
                for k in (2, 3):
                    gpsimd.wait_ge(sA[k], 16)
                    gpsimd.tensor_scalar(
                        ch(m1, k), t1c(k), wcol(k, 0), wcol(k, 1),
                        AOP.mult, AOP.add,
                    ).then_inc(sm, 1)

        @block.vector
        def _(vector):
            vector.memset(ones[:], 1.0)
            vector.wait_ge(swt, 16)
            for k in range(2):
                vector.wait_ge(sA[k], 16)
                vector.tensor_scalar(
                    ch(m1, k), t1c(k), wcol(k, 0), wcol(k, 1), AOP.mult, AOP.add
                )
            if not POOL_M1:
                for k in (2, 3):
                    vector.wait_ge(sA[k], 16)
                    vector.tensor_scalar(
                        ch(m1, k), t1c(k), wcol(k, 0), wcol(k, 1),
                        AOP.mult, AOP.add,
                    )
            vector.wait_ge(sb, 2)
            vector.tensor_add(ch(ss, 0, 2), ch(m1, 0, 2), ch(bb, 0, 2))
            vector.wait_ge(st0[0], 16)
            vector.tensor_mul(ch(pp, 0, 2), ch(t0, 0, 2), ch(ss, 0, 2)).then_inc(
                sv, 1
            )
            vector.wait_ge(sb, 4)
            if POOL_M1:
                vector.wait_ge(sm, 2)
            vector.tensor_add(ch(ss, 2, 2), ch(m1, 2, 2), ch(bb, 2, 2))
            vector.wait_ge(st0[1], 16)
            # split the tail so the last PE matmul starts one op earlier
            vector.tensor_mul(ch(pp, 2), ch(t0, 2), ch(ss, 2)).then_inc(sv, 1)
            vector.tensor_mul(ch(pp, 3), ch(t0, 3), ch(ss, 3)).then_inc(sv, 1)

        @block.tensor
        def _(tensor):
            tensor.wait_ge(sv, 1)
            tensor.matmul(acc[:], ones[:], ch(pp, 0), start=True, stop=False)
            tensor.matmul(acc[:], ones[:], ch(pp, 1), start=False, stop=False)
            tensor.wait_ge(sv, 2)
            tensor.matmul(acc[:], ones[:], ch(pp, 2), start=False, stop=False)
            tensor.wait_ge(sv, 3)
            tensor.matmul(acc[:], ones[:], ch(pp, 3), start=False, stop=True).then_inc(
                sp, 1
            )

    return nc


_NC_CACHE = None


def _get_nc():
    global _NC_CACHE
    if _NC_CACHE is None:
        _NC_CACHE = _build_bass()
    return _NC_CACHE


def _prep_inputs(x, unary, binary, mask):
    """Host-side shard prep: masked diagonals + pre-chunked spin tiles."""
    wr = np.zeros(N, np.float32)
    wd = np.zeros(N, np.float32)
    wr[: N - 1] = np.diagonal(binary, 1) * np.diagonal(mask, 1)
    wd[: N - NG] = np.diagonal(binary, NG) * np.diagonal(mask, NG)
    u = np.asarray(unary, np.float32)

    PADROWS = N + NG + 1
    xt = np.zeros((PADROWS, B), np.float32)
    xt[:N] = np.asarray(x, np.float32).T
    xt16 = xt.astype(NP_FP16)
    xt8 = xt.astype(NP_FP8)

    in_maps = []
    for c in range(NCORES):
        base = c * S
        w = np.empty((128, 3 * NCHUNK), np.float32)
        bufA = np.empty((128, 2 * NCHUNK, B), NP_FP8)
        bufT0 = np.empty((128, NCHUNK, B), NP_FP16)
        for k in range(NCHUNK):
            r0 = base + k * 128
            w[:, 3 * k + 0] = wr[r0 : r0 + 128]
            w[:, 3 * k + 1] = u[r0 : r0 + 128]
            w[:, 3 * k + 2] = wd[r0 : r0 + 128]
            bufA[:, 2 * k] = xt8[r0 + NG : r0 + NG + 128]   # t64 chunk k
            bufA[:, 2 * k + 1] = xt8[r0 + 1 : r0 + 1 + 128]  # t1 chunk k
            bufT0[:, k] = xt16[r0 : r0 + 128]                # t0 chunk k
        in_maps.append(
            {
                "bufA": bufA.reshape(128, 2 * NCHUNK * B),
                "bufT0": bufT0.reshape(128, NCHUNK * B),
                "wts": w,
            }
        )
    return in_maps


def kernel(x, unary, binary, mask):
    nc = _get_nc()
    in_maps = _prep_inputs(x, unary, binary, mask)
    res = run_bass_kernel_spmd(nc, in_maps, list(range(NCORES))).results
    parts = np.stack([np.asarray(r["out"], np.float32) for r in res])  # [8,1,B]
    return parts.sum(axis=(0, 1), dtype=np.float64).astype(np.float32)
